# revision 1
# baseline (speedup 1.0000x reference)
"""Discounted cumsum (B,H,S,D)=(8,16,4096,128), gamma per head, scan along S.

Strategy: batch-parallel across 8 NeuronCores (1 batch each, all 16 heads).
Per head, a two-level chunked scan implemented with PE matmuls (f32r):
  - block size T=128 along S -> 32 blocks per head, processed 4-at-a-time
    (tiles of [128 part = row-in-block, 512 free = 4 blocks x 128 d]).
  - s_k = w^T X_k   (block discounted sums)       [8 matmuls, N=512]
  - c   = AB @ s    (block-level scan, 32x32)     [1 matmul]
  - Y_k = A @ X_k + gvec (x) c_k                  [8+8 matmuls, N=512]
All matmul operands are float32r (1 cyc/row at N>=512); accumulation fp32.
"""
import sys

sys.path.insert(0, "/opt/trn_rl_repo")
import numpy as np

B, H, S, D = 8, 16, 4096, 128
T = 128          # block length along S
KB = S // T      # 32 blocks per head
TILE = 4 * T     # 512 free columns = 4 blocks per matmul
NT = S // TILE   # 8 tiles per head
SKEW_C = 1       # head-pipeline skew for the carry stage
SKEW_B = 3       # head-pipeline skew for the output stage

_CACHE = {}


def _build(repeat=1, mode="full"):
    import contextlib

    import concourse.bacc as bacc
    import concourse.tile as tile
    from concourse import mybir

    f32 = mybir.dt.float32
    f32r = mybir.dt.float32r

    nc = bacc.Bacc("TRN2", target_bir_lowering=False, debug=False)

    x_in = nc.declare_dram_parameter("x", [H, S, D], f32r, isOutput=False)
    at_in = nc.declare_dram_parameter("at", [T, H * T], f32r, isOutput=False)
    w_in = nc.declare_dram_parameter("w", [T, H], f32r, isOutput=False)
    gv_in = nc.declare_dram_parameter("gv", [1, H * T], f32r, isOutput=False)
    abt_in = nc.declare_dram_parameter("abt", [KB, H * KB], f32r, isOutput=False)
    y_out = nc.declare_dram_parameter("y", [H, S, D], f32, isOutput=True)

    with tile.TileContext(nc) as tc:
        with (
            tc.tile_pool(name="const", bufs=1) as const_pool,
            tc.tile_pool(name="xp", bufs=5) as x_pool,
            tc.tile_pool(name="op", bufs=2) as out_pool,
            tc.tile_pool(name="small", bufs=3) as small_pool,
            tc.tile_pool(name="sstage", bufs=2) as sstage_pool,
            tc.tile_pool(name="cflp", bufs=2) as cfl_pool,
            tc.tile_pool(name="sps", bufs=3, space="PSUM") as s_psum,
            tc.tile_pool(name="cps", bufs=1, space="PSUM") as c_psum,
            tc.tile_pool(name="yps", bufs=4, space="PSUM") as y_psum,
        ):
            at_sb = const_pool.tile([T, H * T], f32r)
            w_sb = const_pool.tile([T, H], f32r)
            gv_sb = const_pool.tile([1, H * T], f32r)
            abt_sb = const_pool.tile([KB, H * KB], f32r)
            nc.sync.dma_start(out=at_sb[:], in_=at_in[:])
            nc.sync.dma_start(out=w_sb[:], in_=w_in[:])
            nc.sync.dma_start(out=gv_sb[:], in_=gv_in[:])
            nc.sync.dma_start(out=abt_sb[:], in_=abt_in[:])

            xt = [None] * H      # per-head X tiles [128, 4096], free = (block, d)
            yt = [None] * H      # per-head output staging [128, 4096]
            s32 = [None] * H     # S as [KB, D]
            c32 = [None] * H     # C as [KB, D]
            cfl = [None] * H     # C_flat [1, KB*D]

            def stage_in(h):
                xt[h] = x_pool.tile([T, S], f32r, name=f"xt{h}", tag="xt")
                src = x_in[h].rearrange("(hf k p) d -> hf p k d", k=KB // 2, p=T)
                for hf in range(2):
                    dst = xt[h][:, hf * 2048 : (hf + 1) * 2048].rearrange(
                        "p (k d) -> p k d", d=D
                    )
                    nc.sync.dma_start(out=dst, in_=src[hf])

            def stage_s(h):
                s32[h] = small_pool.tile([KB, D], f32r, name=f"s32{h}", tag="s32")
                s_fl = sstage_pool.tile([1, KB * D], f32r, name="sfl", tag="sfl")
                for t in range(NT):
                    s_ps = s_psum.tile([1, TILE], mybir.dt.float32, name="sps", tag="sps")
                    nc.tensor.matmul(
                        s_ps[:],
                        w_sb[:, h : h + 1],
                        xt[h][:, t * TILE : (t + 1) * TILE],
                        start=True,
                        stop=True,
                    )
                    nc.scalar.copy(
                        out=s_fl[0:1, t * TILE : (t + 1) * TILE], in_=s_ps[:]
                    )
                nc.gpsimd.dma_start(out=s32[h][:], in_=s_fl[:])

            def stage_c(h):
                c_ps = c_psum.tile([KB, D], mybir.dt.float32, name="cps", tag="cps")
                nc.tensor.matmul(
                    c_ps[:],
                    abt_sb[:, h * KB : (h + 1) * KB],
                    s32[h][:],
                    start=True,
                    stop=True,
                )
                c32[h] = small_pool.tile([KB, D], f32r, name=f"c32{h}", tag="c32")
                nc.scalar.copy(out=c32[h][:], in_=c_ps[:])
                cfl[h] = cfl_pool.tile([1, KB * D], f32r, name=f"cf{h}", tag="cf")
                nc.gpsimd.dma_start(out=cfl[h][:], in_=c32[h][:])

            def stage_b(h):
                yt[h] = out_pool.tile([T, S], mybir.dt.float32, name=f"yt{h}", tag="yt")
                for t in range(NT):
                    y_ps = y_psum.tile(
                        [T, TILE], mybir.dt.float32, name="yps", tag="yps"
                    )
                    nc.tensor.matmul(
                        y_ps[:],
                        at_sb[:, h * T : (h + 1) * T],
                        xt[h][:, t * TILE : (t + 1) * TILE],
                        start=True,
                        stop=(mode == "nocarry"),
                    )
                    if mode != "nocarry":
                        nc.tensor.matmul(
                            y_ps[:],
                            gv_sb[0:1, h * T : (h + 1) * T],
                            cfl[h][0:1, t * TILE : (t + 1) * TILE],
                            start=False,
                            stop=True,
                        )
                    nc.vector.tensor_copy(
                        out=yt[h][:, t * TILE : (t + 1) * TILE], in_=y_ps[:]
                    )
                    if t % 2 == 1 and mode != "computeonly":
                        q = t // 2
                        dst = y_out[h].rearrange("(q k p) d -> q p k d", k=NT, p=T)[q]
                        src = yt[h][:, q * 1024 : (q + 1) * 1024].rearrange(
                            "p (k d) -> p k d", d=D
                        )
                        nc.scalar.dma_start(out=dst, in_=src)

            def stage_dma_out(h):
                # store xt straight back (bitcast f32r view of y): DMA floor probe
                for q in range(4):
                    dst = y_out[h].rearrange("(q k p) d -> q p k d", k=NT, p=T)[
                        q
                    ].bitcast(f32r)
                    src = xt[h][:, q * 1024 : (q + 1) * 1024].rearrange(
                        "p (k d) -> p k d", d=D
                    )
                    nc.scalar.dma_start(out=dst, in_=src)

            if mode == "computeonly":
                xconst = const_pool.tile([T, S], f32r)
                nc.vector.memset(xconst[:].bitcast(f32), 0.125)

                def stage_in(h):  # noqa: F811
                    xt[h] = xconst

                def _no_store(h, q):
                    return

            loop = tc.For_i(0, repeat, 1) if repeat > 1 else contextlib.nullcontext()
            with loop:
                if mode == "dmaonly":
                    for i in range(H):
                        stage_in(i)
                        stage_dma_out(i)
                elif mode == "nocarry":
                    for i in range(H + 1):
                        if i < H:
                            stage_in(i)
                        if 0 <= i - 1 < H:
                            stage_b(i - 1)
                else:
                    for i in range(H + SKEW_B):
                        if i < H:
                            stage_in(i)
                            stage_s(i)
                        if 0 <= i - SKEW_C < H:
                            stage_c(i - SKEW_C)
                        if 0 <= i - SKEW_B < H:
                            stage_b(i - SKEW_B)

    nc.compile()
    return nc


def _constants(gamma):
    g = gamma.astype(np.float64)  # [H]
    i = np.arange(T)
    # A_h[i, s] = g^(i-s) for i>=s ; AT[s, h*T+i] = A_h[i, s]
    diff = i[:, None] - i[None, :]  # [i, s]
    at = np.zeros((T, H * T), np.float64)
    w = np.zeros((T, H), np.float64)
    gv = np.zeros((1, H * T), np.float64)
    abt = np.zeros((KB, H * KB), np.float64)
    k = np.arange(KB)
    kdiff = k[None, :] - k[:, None] - 1  # [j, k] -> k-1-j
    for h in range(H):
        gh = g[h]
        a_h = np.where(diff >= 0, gh ** np.maximum(diff, 0), 0.0)  # [i, s]
        at[:, h * T : (h + 1) * T] = a_h.T
        w[:, h] = gh ** (T - 1 - i)
        gv[0, h * T : (h + 1) * T] = gh ** (i + 1)
        G = gh ** T
        abt[:, h * KB : (h + 1) * KB] = np.where(
            kdiff >= 0, G ** np.maximum(kdiff, 0), 0.0
        )
    return (
        at.astype(np.float32),
        w.astype(np.float32),
        gv.astype(np.float32),
        abt.astype(np.float32),
    )


def _fast_callable(nc):
    """Cached jitted shard_map callable (avoids per-call retrace)."""
    import jax
    from jax.experimental.shard_map import shard_map
    from jax.sharding import Mesh, NamedSharding, PartitionSpec
    from concourse import bass2jax, mybir

    bass2jax.install_neuronx_cc_hook()
    partition_name = nc.partition_id_tensor.name if nc.partition_id_tensor else None
    in_names, out_names, out_avals, zero_outs = [], [], [], []
    for alloc in nc.m.functions[0].allocations:
        if not isinstance(alloc, mybir.MemoryLocationSet):
            continue
        name = alloc.memorylocations[0].name
        if alloc.kind == "ExternalInput":
            if name != partition_name:
                in_names.append(name)
        elif alloc.kind == "ExternalOutput":
            shape = tuple(alloc.tensor_shape)
            dtype = mybir.dt.np(alloc.dtype)
            out_avals.append(jax.core.ShapedArray(shape, dtype))
            out_names.append(name)
            zero_outs.append(np.zeros(shape, dtype))
    n_params = len(in_names)
    all_in = list(in_names) + list(out_names)
    if partition_name is not None:
        all_in.append(partition_name)

    def _body(*args):
        operands = list(args)
        if partition_name is not None:
            operands.append(bass2jax.partition_id_tensor())
        return tuple(
            bass2jax._bass_exec_p.bind(
                *operands,
                out_avals=tuple(out_avals),
                in_names=tuple(all_in),
                out_names=tuple(out_names),
                lowering_input_output_aliases=(),
                sim_require_finite=True,
                sim_require_nnan=True,
                nc=nc,
            )
        )

    devices = jax.devices()[:B]
    mesh = Mesh(np.asarray(devices), ("core",))
    specs = (PartitionSpec("core"),)
    f = jax.jit(
        shard_map(
            _body,
            mesh=mesh,
            in_specs=specs * (n_params + len(out_names)),
            out_specs=specs * len(out_names),
            check_rep=False,
        ),
        keep_unused=True,
    )
    sharding = NamedSharding(mesh, PartitionSpec("core"))
    dev_zero = [
        jax.device_put(np.zeros((B * z.shape[0], *z.shape[1:]), z.dtype), sharding)
        for z in zero_outs
    ]
    return f, in_names, out_names, out_avals, sharding, dev_zero


def _run_fast(nc, in_maps):
    import jax

    if "fast" not in _CACHE:
        _CACHE["fast"] = _fast_callable(nc)
    f, in_names, out_names, out_avals, sharding, dev_zero = _CACHE["fast"]
    concat_in = [
        jax.device_put(
            np.concatenate([np.asarray(m[nm]) for m in in_maps], axis=0), sharding
        )
        for nm in in_names
    ]
    outs = f(*concat_in, *dev_zero)
    return [
        {
            nm: np.asarray(outs[i]).reshape(B, *out_avals[i].shape)[c]
            for i, nm in enumerate(out_names)
        }
        for c in range(B)
    ]


def _run(tensor, gamma, trace=False, repeat=1):
    from concourse.bass_utils import run_bass_kernel_spmd

    key = f"nc{repeat}"
    if key not in _CACHE:
        _CACHE[key] = _build(repeat)
    nc = _CACHE[key]

    at, w, gv, abt = _constants(np.asarray(gamma))
    tensor = np.asarray(tensor, dtype=np.float32)
    in_maps = [
        {
            "x": np.ascontiguousarray(tensor[c]),
            "at": at,
            "w": w,
            "gv": gv,
            "abt": abt,
        }
        for c in range(B)
    ]
    if repeat == 1 and not trace:
        try:
            results = _run_fast(nc, in_maps)
            y = np.stack([results[c]["y"] for c in range(B)], axis=0)
            return y, None
        except Exception:
            pass  # fall back to the reference path below
    res = run_bass_kernel_spmd(nc, in_maps, core_ids=list(range(B)), trace=trace)
    y = np.stack([res.results[c]["y"] for c in range(B)], axis=0)
    return y, res


def kernel(tensor, gamma):
    try:
        y, _ = _run(tensor, gamma)
    except Exception:
        # transient device/pool errors: clear cached state and retry once
        _CACHE.clear()
        y, _ = _run(tensor, gamma)
    return y



# revision 9
# speedup vs baseline: 2.4696x; 2.4696x over previous
"""Discounted cumsum (B,H,S,D)=(8,16,4096,128), gamma per head, scan along S.

Strategy: batch-parallel across 8 NeuronCores (1 batch each, all 16 heads).
Device IO is bf16 with a host-side layout permute; heads are PAIRED per
DMA so every HBM transfer is [128 partitions x 16.5 KiB] — fewer, larger
descriptors (the DMA subsystem is descriptor-throughput-bound, and odd
partition counts fall off a 6x cliff, so always 128 partitions).

Blocks are Tb=127 long so the per-block carry rides in the 128th
contraction row of a single fused matmul:
  x DRAM [H/2, 128, 2*KB*D]: row p = position-in-block p of both heads of
  the pair ((k d) packed per head); row 127 is zero, filled on-device with
  the block carries C_k.

Per head:
  - s_k = X_k^T w  via 33 weight-load matmuls (N=1) -> sT [128(d), 33] PSUM,
    one cheap copy, PE transpose, copy -> s32 [33, 128] bf16.
  - c = ABt^T @ s32  (block-level scan, 33x33)  [1 matmul]; c -> xt row 127.
  - Y_k = A @ X_k + gvec (x) C_k  in ONE fused matmul per 4-block tile
    (lhsT = [aT ; gv], K=128 = 127 x-rows + carry row; M=128 where output
    column 127 = [w ; Gn] emits the next-block carries, initializing yt
    row 127 for the padded store).
Matmul operands bf16, accumulation fp32 in PSUM; y stored bf16 and upcast
to f32 on the host (rel-err budget 2e-2 >> bf16's ~4e-3).
"""
import sys

sys.path.insert(0, "/opt/trn_rl_repo")
import ml_dtypes
import numpy as np

BF16 = ml_dtypes.bfloat16
B, H, S, D = 8, 16, 4096, 128
TB = 127         # block length along S (127 so carry rides in row 128)
KB = 33          # ceil(S / TB) blocks per head (last block partial)
FD = KB * D      # 4224 free columns per head
HP = H // 2      # head pairs
PF = 2 * FD      # 8448 free columns per pair tile
TILE = 4 * D     # 512 free columns = 4 blocks per matmul
NT = 8           # full tiles per head (tile 8 is the 1-block tail)
SKEW_S = 2       # block sums lag the pair input DMA
SKEW_C = 3       # carry scan lags block sums
SKEW_B = 5       # output stage lags carry

_CACHE = {}


def _build(repeat=1, mode="full"):
    import contextlib

    import concourse.bacc as bacc
    import concourse.tile as tile
    from concourse import mybir

    f32 = mybir.dt.float32
    bf16 = mybir.dt.bfloat16

    nc = bacc.Bacc("TRN2", target_bir_lowering=False, debug=False)

    x_in = nc.declare_dram_parameter("x", [HP, 128, PF], bf16, isOutput=False)
    atg_in = nc.declare_dram_parameter("atg", [128, H * 128], bf16, isOutput=False)
    w_in = nc.declare_dram_parameter("w", [TB, H], bf16, isOutput=False)
    abt_in = nc.declare_dram_parameter("abt", [KB, H * KB], bf16, isOutput=False)
    id_in = nc.declare_dram_parameter("idm", [128, 128], bf16, isOutput=False)
    y_out = nc.declare_dram_parameter("y", [HP, 128, PF], bf16, isOutput=True)

    with tile.TileContext(nc) as tc:
        with (
            tc.tile_pool(name="const", bufs=1) as const_pool,
            tc.tile_pool(name="xp", bufs=4) as x_pool,
            tc.tile_pool(name="op", bufs=2) as out_pool,
            tc.tile_pool(name="small", bufs=4) as small_pool,
            tc.tile_pool(name="sstage", bufs=2) as sstage_pool,
            tc.tile_pool(name="stps", bufs=1, space="PSUM") as st_psum,
            tc.tile_pool(name="s32ps", bufs=1, space="PSUM") as s32_psum,
            tc.tile_pool(name="cps", bufs=1, space="PSUM") as c_psum,
            tc.tile_pool(name="yps", bufs=5, space="PSUM") as y_psum,
        ):
            atg_sb = const_pool.tile([128, H * 128], bf16)
            w_sb = const_pool.tile([TB, H], bf16)
            abt_sb = const_pool.tile([KB, H * KB], bf16)
            id_sb = const_pool.tile([128, 128], bf16)
            nc.sync.dma_start(out=atg_sb[:], in_=atg_in[:])
            nc.sync.dma_start(out=w_sb[:], in_=w_in[:])
            nc.sync.dma_start(out=abt_sb[:], in_=abt_in[:])
            nc.sync.dma_start(out=id_sb[:], in_=id_in[:])

            xt = [None] * HP     # pair tiles [128, PF]; row 127 = carries
            yt = [None] * HP     # pair output staging [128, PF]
            s32 = [None] * H     # block sums [KB, D]

            def stage_in(h):
                j = h // 2
                xt[j] = x_pool.tile([128, PF], bf16, name=f"xt{j}", tag="xt")
                nc.sync.dma_start(out=xt[j][:], in_=x_in[j])

            def stage_s(h):
                j, c0 = h // 2, (h % 2) * FD
                # sT[d, k] = sum_p X_k[p, d] w[p]: one weight-load matmul per
                # block, all N=1 into one [128, KB] PSUM tile.
                st_ps = st_psum.tile([128, KB], f32, name="stps", tag="stps")
                for k in range(KB):
                    nc.tensor.matmul(
                        st_ps[:, k : k + 1],
                        xt[j][0:TB, c0 + k * D : c0 + (k + 1) * D],
                        w_sb[:, h : h + 1],
                        start=True,
                        stop=True,
                    )
                st_sb = sstage_pool.tile([128, KB], bf16, name="stsb", tag="stsb")
                nc.vector.tensor_copy(out=st_sb[:], in_=st_ps[:])
                s32_ps = s32_psum.tile([KB, 128], bf16, name="s32p", tag="s32p")
                nc.tensor.transpose(s32_ps[:], st_sb[:], id_sb[:])
                s32[h] = small_pool.tile([KB, 128], bf16, name=f"s32{h}", tag="s32")
                nc.scalar.copy(out=s32[h][:], in_=s32_ps[:])

            def stage_c(h):
                j, c0 = h // 2, (h % 2) * FD
                c_ps = c_psum.tile([KB, D], f32, name="cps", tag="cps")
                nc.tensor.matmul(
                    c_ps[:],
                    abt_sb[:, h * KB : (h + 1) * KB],
                    s32[h][:],
                    start=True,
                    stop=True,
                )
                c32 = small_pool.tile([KB, D], bf16, name=f"c32{h}", tag="c32")
                nc.scalar.copy(out=c32[:], in_=c_ps[:])
                # carry row: C_k lands in xt row 127 at free (k d); split into
                # an even-partition-count chunk + remainder (odd counts hit a
                # slow descriptor-gen path), issued from the scalar queue so
                # they order naturally after the c32 copy.
                if mode == "full":
                    dst = xt[j][TB : TB + 1, c0 : c0 + FD]
                else:  # probe modes: same DMA cost, no xt dependency
                    scr = small_pool.tile([1, FD], bf16, name="scr", tag="scr")
                    dst = scr[0:1, :]
                nc.scalar.dma_start(out=dst[0:1, 0 : 32 * D], in_=c32[0:32, :])
                nc.scalar.dma_start(out=dst[0:1, 32 * D : FD], in_=c32[32:33, :])

            def stage_b(h):
                j, c0 = h // 2, (h % 2) * FD
                if h % 2 == 0:
                    yt[j] = out_pool.tile([128, PF], bf16, name=f"yt{j}", tag="yt")
                kk = TB if mode == "nocarry" else 128
                for t in range(NT + 1):
                    n = TILE if t < NT else D
                    y_ps = y_psum.tile([128, TILE], f32, name="yps", tag="yps")
                    nc.tensor.matmul(
                        y_ps[:, 0:n],
                        atg_sb[0:kk, h * 128 : (h + 1) * 128],
                        xt[j][0:kk, c0 + t * TILE : c0 + t * TILE + n],
                        start=True,
                        stop=True,
                    )
                    if t % 2 == 0:
                        nc.vector.tensor_copy(
                            out=yt[j][:, c0 + t * TILE : c0 + t * TILE + n],
                            in_=y_ps[:, 0:n],
                        )
                    else:
                        nc.scalar.copy(
                            out=yt[j][:, c0 + t * TILE : c0 + t * TILE + n],
                            in_=y_ps[:, 0:n],
                        )
                if h % 2 == 1 and mode != "computeonly":
                    nc.gpsimd.dma_start(out=y_out[j], in_=yt[j][:])

            def stage_dma_out(h):
                # store xt straight back: DMA floor probe
                j = h // 2
                nc.gpsimd.dma_start(out=y_out[j], in_=xt[j][:])

            if mode in ("computeonly", "noin"):
                xconst = const_pool.tile([128, PF], bf16)
                nc.vector.memset(xconst[:], 0.125)

                def stage_in(h):  # noqa: F811
                    xt[h // 2] = xconst

            loop = tc.For_i(0, repeat, 1) if repeat > 1 else contextlib.nullcontext()
            with loop:
                if mode == "dmaonly":
                    for i in range(0, H, 2):
                        stage_in(i)
                        stage_dma_out(i)
                else:
                    for i in range(H + SKEW_B):
                        if i < H and i % 2 == 0:
                            stage_in(i)
                        if 0 <= i - SKEW_B < H:
                            stage_b(i - SKEW_B)
                        if mode != "nocarry":
                            if 0 <= i - SKEW_S < H:
                                stage_s(i - SKEW_S)
                            if 0 <= i - SKEW_C < H:
                                stage_c(i - SKEW_C)

    nc.compile()
    return nc


def _constants(gamma):
    g = gamma.astype(np.float64)  # [H]
    m = np.arange(TB)
    diff = m[:, None] - m[None, :]  # [m, p']
    atg = np.zeros((128, H * 128), np.float64)
    w = np.zeros((TB, H), np.float64)
    abt = np.zeros((KB, H * KB), np.float64)
    k = np.arange(KB)
    kdiff = k[None, :] - k[:, None] - 1  # [j, k] -> k-1-j
    for h in range(H):
        gh = g[h]
        Gn = gh ** TB
        # output rows m=0..126: col block [p', m] = g^(m-p') for m>=p',
        # carry row (p'=127): g^(m+1).  Output col 127 = next-carry row:
        # [p', 127] = w[p'] = g^(126-p'), [127, 127] = Gn.
        a_h = np.where(diff >= 0, gh ** np.maximum(diff, 0), 0.0)  # [m, p']
        atg[0:TB, h * 128 : h * 128 + TB] = a_h.T
        atg[TB, h * 128 : h * 128 + TB] = gh ** (m + 1)
        atg[0:TB, h * 128 + TB] = gh ** (TB - 1 - m)
        atg[TB, h * 128 + TB] = Gn
        w[:, h] = gh ** (TB - 1 - m)
        abt[:, h * KB : (h + 1) * KB] = np.where(
            kdiff >= 0, Gn ** np.maximum(kdiff, 0), 0.0
        )
    idm = np.eye(128, dtype=np.float64)
    return (
        atg.astype(BF16),
        w.astype(BF16),
        abt.astype(BF16),
        idm.astype(BF16),
    )


def _prepare(tensor, gamma):
    """Host-side prep: bf16 cast + pad + permute + head-pair packing."""
    atg, w, abt, idm = _constants(np.asarray(gamma))
    xb = np.asarray(tensor, dtype=np.float32).astype(BF16)  # [B,H,S,D]
    in_maps = []
    for c in range(B):
        xpad = np.zeros((H, KB * TB, D), BF16)
        xpad[:, :S] = xb[c]
        perm = np.ascontiguousarray(
            xpad.reshape(H, KB, TB, D).transpose(0, 2, 1, 3)
        ).reshape(H, TB, FD)
        xp = np.zeros((HP, 128, PF), BF16)
        xp[:, :TB, :FD] = perm[0::2]
        xp[:, :TB, FD:] = perm[1::2]
        in_maps.append({"x": xp, "atg": atg, "w": w, "abt": abt, "idm": idm})
    return in_maps


def _postprocess(y_dev):
    """[HP, 128, PF] bf16 device layout -> [H, S, D] f32."""
    arr = np.stack([y_dev[:, :TB, :FD], y_dev[:, :TB, FD:]], axis=1)
    return (
        arr.astype(np.float32)
        .reshape(H, TB, KB, D)
        .transpose(0, 2, 1, 3)
        .reshape(H, KB * TB, D)[:, :S]
    )


def _fast_callable(nc):
    """Cached jitted shard_map callable (avoids per-call retrace)."""
    import jax
    from jax.experimental.shard_map import shard_map
    from jax.sharding import Mesh, NamedSharding, PartitionSpec
    from concourse import bass2jax, mybir

    bass2jax.install_neuronx_cc_hook()
    partition_name = nc.partition_id_tensor.name if nc.partition_id_tensor else None
    in_names, out_names, out_avals, zero_outs = [], [], [], []
    for alloc in nc.m.functions[0].allocations:
        if not isinstance(alloc, mybir.MemoryLocationSet):
            continue
        name = alloc.memorylocations[0].name
        if alloc.kind == "ExternalInput":
            if name != partition_name:
                in_names.append(name)
        elif alloc.kind == "ExternalOutput":
            shape = tuple(alloc.tensor_shape)
            dtype = mybir.dt.np(alloc.dtype)
            out_avals.append(jax.core.ShapedArray(shape, dtype))
            out_names.append(name)
            zero_outs.append(np.zeros(shape, dtype))
    n_params = len(in_names)
    all_in = list(in_names) + list(out_names)
    if partition_name is not None:
        all_in.append(partition_name)

    def _body(*args):
        operands = list(args)
        if partition_name is not None:
            operands.append(bass2jax.partition_id_tensor())
        return tuple(
            bass2jax._bass_exec_p.bind(
                *operands,
                out_avals=tuple(out_avals),
                in_names=tuple(all_in),
                out_names=tuple(out_names),
                lowering_input_output_aliases=(),
                sim_require_finite=True,
                sim_require_nnan=True,
                nc=nc,
            )
        )

    devices = jax.devices()[:B]
    mesh = Mesh(np.asarray(devices), ("core",))
    specs = (PartitionSpec("core"),)
    f = jax.jit(
        shard_map(
            _body,
            mesh=mesh,
            in_specs=specs * (n_params + len(out_names)),
            out_specs=specs * len(out_names),
            check_rep=False,
        ),
        keep_unused=True,
    )
    sharding = NamedSharding(mesh, PartitionSpec("core"))
    dev_zero = [
        jax.device_put(np.zeros((B * z.shape[0], *z.shape[1:]), z.dtype), sharding)
        for z in zero_outs
    ]
    return f, in_names, out_names, out_avals, sharding, dev_zero


def _run_fast(nc, in_maps):
    import jax

    if "fast" not in _CACHE:
        _CACHE["fast"] = _fast_callable(nc)
    f, in_names, out_names, out_avals, sharding, dev_zero = _CACHE["fast"]
    concat_in = [
        jax.device_put(
            np.concatenate([np.asarray(m[nm]) for m in in_maps], axis=0), sharding
        )
        for nm in in_names
    ]
    outs = f(*concat_in, *dev_zero)
    return [
        {
            nm: np.asarray(outs[i]).reshape(B, *out_avals[i].shape)[c]
            for i, nm in enumerate(out_names)
        }
        for c in range(B)
    ]


def _run(tensor, gamma, trace=False, repeat=1):
    from concourse.bass_utils import run_bass_kernel_spmd

    key = f"nc{repeat}"
    if key not in _CACHE:
        _CACHE[key] = _build(repeat)
    nc = _CACHE[key]

    in_maps = _prepare(tensor, gamma)
    if repeat == 1 and not trace:
        try:
            results = _run_fast(nc, in_maps)
            y = np.stack([_postprocess(results[c]["y"]) for c in range(B)], axis=0)
            return y, None
        except Exception:
            pass  # fall back to the reference path below
    res = run_bass_kernel_spmd(nc, in_maps, core_ids=list(range(B)), trace=trace)
    y = np.stack([_postprocess(res.results[c]["y"]) for c in range(B)], axis=0)
    return y, res


def kernel(tensor, gamma):
    try:
        y, _ = _run(tensor, gamma)
    except Exception:
        # transient device/pool errors: clear cached state and retry once
        _CACHE.clear()
        y, _ = _run(tensor, gamma)
    return y


# revision 12
# speedup vs baseline: 2.8440x; 1.1516x over previous
"""Discounted cumsum (B,H,S,D)=(8,16,4096,128), gamma per head, scan along S.

Strategy: batch-parallel across 8 NeuronCores (1 batch each, all 16 heads).
Device IO is bf16 with a host-side layout permute; heads are PAIRED per
DMA so every HBM transfer is [128 partitions x 16.5 KiB] — fewer, larger
descriptors (the DMA subsystem is descriptor-throughput-bound, and odd
partition counts fall off a 6x cliff, so always 128 partitions).

Blocks are Tb=127 long so the per-block carry rides in the 128th
contraction row of a single fused matmul:
  x DRAM [H/2, 128, 2*KB*D]: row p = position-in-block p of both heads of
  the pair ((k d) packed per head); row 127 is zero, filled on-device with
  the block carries C_k.

Per head:
  - s_k = X_k^T w  via 33 weight-load matmuls (N=1) -> sT [128(d), 33] PSUM,
    one cheap copy, PE transpose, copy -> s32 [33, 128] bf16.
  - c = ABt^T @ s32  (block-level scan, 33x33)  [1 matmul]; c -> xt row 127.
  - Y_k = A @ X_k + gvec (x) C_k  in ONE fused matmul per 4-block tile
    (lhsT = [aT ; gv], K=128 = 127 x-rows + carry row; M=128 where output
    column 127 = [w ; Gn] emits the next-block carries, initializing yt
    row 127 for the padded store).
Matmul operands bf16, accumulation fp32 in PSUM; y stored bf16 and upcast
to f32 on the host (rel-err budget 2e-2 >> bf16's ~4e-3).
"""
import sys

sys.path.insert(0, "/opt/trn_rl_repo")
import ml_dtypes
import numpy as np

BF16 = ml_dtypes.bfloat16
B, H, S, D = 8, 16, 4096, 128
TB = 127         # block length along S (127 so carry rides in row 128)
KB = 33          # ceil(S / TB) blocks per head (last block partial)
FD = KB * D      # 4224 free columns per head
HP = H // 2      # head pairs
PF = 2 * FD      # 8448 free columns per pair tile
TILE = 4 * D     # 512 free columns = 4 blocks per matmul
NT = 8           # full tiles per head (tile 8 is the 1-block tail)
SKEW_S = 2       # block sums lag the pair input DMA
SKEW_C = 3       # carry scan lags block sums
SKEW_B = 4       # output stage lags carry

_CACHE = {}


def _build(repeat=1, mode="full"):
    import contextlib

    import concourse.bacc as bacc
    import concourse.tile as tile
    from concourse import mybir

    f32 = mybir.dt.float32
    bf16 = mybir.dt.bfloat16

    nc = bacc.Bacc("TRN2", target_bir_lowering=False, debug=False)

    x_in = nc.declare_dram_parameter("x", [HP, 128, PF], bf16, isOutput=False)
    atg_in = nc.declare_dram_parameter("atg", [128, H * 128], bf16, isOutput=False)
    w_in = nc.declare_dram_parameter("w", [TB, H], bf16, isOutput=False)
    abt_in = nc.declare_dram_parameter("abt", [KB, H * KB], bf16, isOutput=False)
    id_in = nc.declare_dram_parameter("idm", [128, 128], bf16, isOutput=False)
    y_out = nc.declare_dram_parameter("y", [HP, 128, PF], bf16, isOutput=True)

    with tile.TileContext(nc) as tc:
        with (
            tc.tile_pool(name="const", bufs=1) as const_pool,
            tc.tile_pool(name="xp", bufs=4) as x_pool,
            tc.tile_pool(name="op", bufs=2) as out_pool,
            tc.tile_pool(name="small", bufs=4) as small_pool,
            tc.tile_pool(name="sstage", bufs=2) as sstage_pool,
            tc.tile_pool(name="stps", bufs=1, space="PSUM") as st_psum,
            tc.tile_pool(name="s32ps", bufs=1, space="PSUM") as s32_psum,
            tc.tile_pool(name="cps", bufs=1, space="PSUM") as c_psum,
            tc.tile_pool(name="yps", bufs=5, space="PSUM") as y_psum,
        ):
            atg_sb = const_pool.tile([128, H * 128], bf16)
            w_sb = const_pool.tile([TB, H], bf16)
            abt_sb = const_pool.tile([KB, H * KB], bf16)
            id_sb = const_pool.tile([128, 128], bf16)
            nc.sync.dma_start(out=atg_sb[:], in_=atg_in[:])
            nc.sync.dma_start(out=w_sb[:], in_=w_in[:])
            nc.sync.dma_start(out=abt_sb[:], in_=abt_in[:])
            nc.sync.dma_start(out=id_sb[:], in_=id_in[:])

            xt = [None] * HP     # pair tiles [128, PF]; row 127 = carries
            yt = [None] * HP     # pair output staging [128, PF]
            s32 = [None] * H     # block sums [KB, D]

            def stage_in(h):
                j = h // 2
                xt[j] = x_pool.tile([128, PF], bf16, name=f"xt{j}", tag="xt")
                nc.sync.dma_start(out=xt[j][:], in_=x_in[j])

            def stage_s(h):
                j, c0 = h // 2, (h % 2) * FD
                # sT[d, k] = sum_p X_k[p, d] w[p]: one weight-load matmul per
                # block, all N=1 into one [128, KB] PSUM tile.
                st_ps = st_psum.tile([128, KB], f32, name="stps", tag="stps")
                for k in range(KB):
                    nc.tensor.matmul(
                        st_ps[:, k : k + 1],
                        xt[j][0:TB, c0 + k * D : c0 + (k + 1) * D],
                        w_sb[:, h : h + 1],
                        start=True,
                        stop=True,
                    )
                st_sb = sstage_pool.tile([128, KB], bf16, name="stsb", tag="stsb")
                nc.vector.tensor_copy(out=st_sb[:], in_=st_ps[:])
                s32_ps = s32_psum.tile([KB, 128], bf16, name="s32p", tag="s32p")
                nc.tensor.transpose(s32_ps[:], st_sb[:], id_sb[:])
                s32[h] = small_pool.tile([KB, 128], bf16, name=f"s32{h}", tag="s32")
                nc.scalar.copy(out=s32[h][:], in_=s32_ps[:])

            def stage_c(h):
                j, c0 = h // 2, (h % 2) * FD
                c_ps = c_psum.tile([KB, D], f32, name="cps", tag="cps")
                nc.tensor.matmul(
                    c_ps[:],
                    abt_sb[:, h * KB : (h + 1) * KB],
                    s32[h][:],
                    start=True,
                    stop=True,
                )
                c32 = small_pool.tile([KB, D], bf16, name=f"c32{h}", tag="c32")
                nc.scalar.copy(out=c32[:], in_=c_ps[:])
                # carry row: C_k lands in xt row 127 at free (k d); split into
                # an even-partition-count chunk + remainder (odd counts hit a
                # slow descriptor-gen path), issued from the scalar queue so
                # they order naturally after the c32 copy.
                if mode == "full":
                    dst = xt[j][TB : TB + 1, c0 : c0 + FD]
                else:  # probe modes: same DMA cost, no xt dependency
                    scr = small_pool.tile([1, FD], bf16, name="scr", tag="scr")
                    dst = scr[0:1, :]
                nc.sync.dma_start(out=dst[0:1, 0 : 32 * D], in_=c32[0:32, :])
                nc.sync.dma_start(out=dst[0:1, 32 * D : FD], in_=c32[32:33, :])

            def stage_b(h):
                j, c0 = h // 2, (h % 2) * FD
                if h % 2 == 0:
                    yt[j] = out_pool.tile([128, PF], bf16, name=f"yt{j}", tag="yt")
                kk = TB if mode == "nocarry" else 128
                for t in range(NT + 1):
                    n = TILE if t < NT else D
                    y_ps = y_psum.tile([128, TILE], f32, name="yps", tag="yps")
                    nc.tensor.matmul(
                        y_ps[:, 0:n],
                        atg_sb[0:kk, h * 128 : (h + 1) * 128],
                        xt[j][0:kk, c0 + t * TILE : c0 + t * TILE + n],
                        start=True,
                        stop=True,
                    )
                    if t % 2 == 0:
                        nc.vector.tensor_copy(
                            out=yt[j][:, c0 + t * TILE : c0 + t * TILE + n],
                            in_=y_ps[:, 0:n],
                        )
                    else:
                        nc.scalar.copy(
                            out=yt[j][:, c0 + t * TILE : c0 + t * TILE + n],
                            in_=y_ps[:, 0:n],
                        )
                if mode != "computeonly":
                    # split the pair store into two 512B-aligned halves so the
                    # first fires one head earlier (shrinks pipeline drain)
                    if h % 2 == 0:
                        nc.gpsimd.dma_start(
                            out=y_out[j][:, 0:4096], in_=yt[j][:, 0:4096]
                        )
                    else:
                        nc.gpsimd.dma_start(
                            out=y_out[j][:, 4096:PF], in_=yt[j][:, 4096:PF]
                        )

            def stage_dma_out(h):
                # store xt straight back: DMA floor probe
                j = h // 2
                nc.gpsimd.dma_start(out=y_out[j], in_=xt[j][:])

            if mode in ("computeonly", "noin"):
                xconst = const_pool.tile([128, PF], bf16)
                nc.vector.memset(xconst[:], 0.125)

                def stage_in(h):  # noqa: F811
                    xt[h // 2] = xconst

            loop = tc.For_i(0, repeat, 1) if repeat > 1 else contextlib.nullcontext()
            with loop:
                if mode == "dmaonly":
                    for i in range(0, H, 2):
                        stage_in(i)
                        stage_dma_out(i)
                else:
                    for i in range(H + SKEW_B):
                        if i < H and i % 2 == 0:
                            stage_in(i)
                        if 0 <= i - SKEW_B < H:
                            stage_b(i - SKEW_B)
                        if mode != "nocarry":
                            if 0 <= i - SKEW_S < H:
                                stage_s(i - SKEW_S)
                            if 0 <= i - SKEW_C < H:
                                stage_c(i - SKEW_C)

    nc.compile()
    return nc


def _constants(gamma):
    g = gamma.astype(np.float64)  # [H]
    m = np.arange(TB)
    diff = m[:, None] - m[None, :]  # [m, p']
    atg = np.zeros((128, H * 128), np.float64)
    w = np.zeros((TB, H), np.float64)
    abt = np.zeros((KB, H * KB), np.float64)
    k = np.arange(KB)
    kdiff = k[None, :] - k[:, None] - 1  # [j, k] -> k-1-j
    for h in range(H):
        gh = g[h]
        Gn = gh ** TB
        # output rows m=0..126: col block [p', m] = g^(m-p') for m>=p',
        # carry row (p'=127): g^(m+1).  Output col 127 = next-carry row:
        # [p', 127] = w[p'] = g^(126-p'), [127, 127] = Gn.
        a_h = np.where(diff >= 0, gh ** np.maximum(diff, 0), 0.0)  # [m, p']
        atg[0:TB, h * 128 : h * 128 + TB] = a_h.T
        atg[TB, h * 128 : h * 128 + TB] = gh ** (m + 1)
        atg[0:TB, h * 128 + TB] = gh ** (TB - 1 - m)
        atg[TB, h * 128 + TB] = Gn
        w[:, h] = gh ** (TB - 1 - m)
        abt[:, h * KB : (h + 1) * KB] = np.where(
            kdiff >= 0, Gn ** np.maximum(kdiff, 0), 0.0
        )
    idm = np.eye(128, dtype=np.float64)
    return (
        atg.astype(BF16),
        w.astype(BF16),
        abt.astype(BF16),
        idm.astype(BF16),
    )


def _prepare(tensor, gamma):
    """Host-side prep: bf16 cast + pad + permute + head-pair packing."""
    atg, w, abt, idm = _constants(np.asarray(gamma))
    xb = np.asarray(tensor, dtype=np.float32).astype(BF16)  # [B,H,S,D]
    in_maps = []
    for c in range(B):
        xpad = np.zeros((H, KB * TB, D), BF16)
        xpad[:, :S] = xb[c]
        perm = np.ascontiguousarray(
            xpad.reshape(H, KB, TB, D).transpose(0, 2, 1, 3)
        ).reshape(H, TB, FD)
        xp = np.zeros((HP, 128, PF), BF16)
        xp[:, :TB, :FD] = perm[0::2]
        xp[:, :TB, FD:] = perm[1::2]
        in_maps.append({"x": xp, "atg": atg, "w": w, "abt": abt, "idm": idm})
    return in_maps


def _postprocess(y_dev):
    """[HP, 128, PF] bf16 device layout -> [H, S, D] f32."""
    arr = np.stack([y_dev[:, :TB, :FD], y_dev[:, :TB, FD:]], axis=1)
    return (
        arr.astype(np.float32)
        .reshape(H, TB, KB, D)
        .transpose(0, 2, 1, 3)
        .reshape(H, KB * TB, D)[:, :S]
    )


def _fast_callable(nc):
    """Cached jitted shard_map callable (avoids per-call retrace)."""
    import jax
    from jax.experimental.shard_map import shard_map
    from jax.sharding import Mesh, NamedSharding, PartitionSpec
    from concourse import bass2jax, mybir

    bass2jax.install_neuronx_cc_hook()
    partition_name = nc.partition_id_tensor.name if nc.partition_id_tensor else None
    in_names, out_names, out_avals, zero_outs = [], [], [], []
    for alloc in nc.m.functions[0].allocations:
        if not isinstance(alloc, mybir.MemoryLocationSet):
            continue
        name = alloc.memorylocations[0].name
        if alloc.kind == "ExternalInput":
            if name != partition_name:
                in_names.append(name)
        elif alloc.kind == "ExternalOutput":
            shape = tuple(alloc.tensor_shape)
            dtype = mybir.dt.np(alloc.dtype)
            out_avals.append(jax.core.ShapedArray(shape, dtype))
            out_names.append(name)
            zero_outs.append(np.zeros(shape, dtype))
    n_params = len(in_names)
    all_in = list(in_names) + list(out_names)
    if partition_name is not None:
        all_in.append(partition_name)

    def _body(*args):
        operands = list(args)
        if partition_name is not None:
            operands.append(bass2jax.partition_id_tensor())
        return tuple(
            bass2jax._bass_exec_p.bind(
                *operands,
                out_avals=tuple(out_avals),
                in_names=tuple(all_in),
                out_names=tuple(out_names),
                lowering_input_output_aliases=(),
                sim_require_finite=True,
                sim_require_nnan=True,
                nc=nc,
            )
        )

    devices = jax.devices()[:B]
    mesh = Mesh(np.asarray(devices), ("core",))
    specs = (PartitionSpec("core"),)
    f = jax.jit(
        shard_map(
            _body,
            mesh=mesh,
            in_specs=specs * (n_params + len(out_names)),
            out_specs=specs * len(out_names),
            check_rep=False,
        ),
        keep_unused=True,
    )
    sharding = NamedSharding(mesh, PartitionSpec("core"))
    dev_zero = [
        jax.device_put(np.zeros((B * z.shape[0], *z.shape[1:]), z.dtype), sharding)
        for z in zero_outs
    ]
    return f, in_names, out_names, out_avals, sharding, dev_zero


def _run_fast(nc, in_maps):
    import jax

    if "fast" not in _CACHE:
        _CACHE["fast"] = _fast_callable(nc)
    f, in_names, out_names, out_avals, sharding, dev_zero = _CACHE["fast"]
    concat_in = [
        jax.device_put(
            np.concatenate([np.asarray(m[nm]) for m in in_maps], axis=0), sharding
        )
        for nm in in_names
    ]
    outs = f(*concat_in, *dev_zero)
    return [
        {
            nm: np.asarray(outs[i]).reshape(B, *out_avals[i].shape)[c]
            for i, nm in enumerate(out_names)
        }
        for c in range(B)
    ]


def _run(tensor, gamma, trace=False, repeat=1):
    from concourse.bass_utils import run_bass_kernel_spmd

    key = f"nc{repeat}"
    if key not in _CACHE:
        _CACHE[key] = _build(repeat)
    nc = _CACHE[key]

    in_maps = _prepare(tensor, gamma)
    if repeat == 1 and not trace:
        try:
            results = _run_fast(nc, in_maps)
            y = np.stack([_postprocess(results[c]["y"]) for c in range(B)], axis=0)
            return y, None
        except Exception:
            pass  # fall back to the reference path below
    res = run_bass_kernel_spmd(nc, in_maps, core_ids=list(range(B)), trace=trace)
    y = np.stack([_postprocess(res.results[c]["y"]) for c in range(B)], axis=0)
    return y, res


def kernel(tensor, gamma):
    try:
        y, _ = _run(tensor, gamma)
    except Exception:
        # transient device/pool errors: clear cached state and retry once
        _CACHE.clear()
        y, _ = _run(tensor, gamma)
    return y
